# revision 35
# baseline (speedup 1.0000x reference)
"""GGNN message-passing encoder on 8 Trainium2 NeuronCores.

Data-parallel over batch B=8: core b processes batch element b end-to-end
(its own [N,N] adjacency slice; small GGNN weights replicated), no
collectives. The whole working set (adjT 16.8 MB + state + weights) lives
in SBUF, so the adjacency is read from HBM exactly once and reused for all
NBLOCKS*NSTEPS message-passing matmuls.

Kernel layout: the state is kept feature-major (hT [D=128 part, N=2048
free]) so every matmul has a 512-wide moving operand and runs at full PE
rate in float32r. The adjacency is transposed on-chip (PE transposes,
grouped 4-per-PSUM-bank) into adjT[m, n] once at load time.
"""

import sys

sys.path.insert(0, "/opt/trn_rl_repo")

from contextlib import ExitStack

import numpy as np

import concourse.bass as bass
import concourse.mybir as mybir
import concourse.tile as tile
from concourse import bacc
from concourse.bass import IndirectOffsetOnAxis
from concourse.masks import make_identity

P = 128
B = 8
NL = 1024
LL = 1024
N = NL + LL          # 2048 nodes
D = 128
V = 50000
NBLOCKS = 5
NSTEPS = 3
NT = N // P          # 16 node tiles
CH = 512             # n-chunk (PSUM bank width in fp32)
NCH = N // CH        # 4 chunks
F32 = mybir.dt.float32
F32R = mybir.dt.float32r
I32 = mybir.dt.int32

N_CORES = 8
COPIES_ON_ACT = True


def _r(ap):
    """View an fp32 AP as float32r for full-rate PE matmuls."""
    return ap.bitcast(F32R)


def build_nc(nblocks=NBLOCKS, nsteps=NSTEPS):
    nc = bacc.Bacc("TRN2", target_bir_lowering=False, debug=False,
                   num_devices=N_CORES)

    # ---- per-core DRAM tensors (each core gets its own batch slice) ----
    adj = nc.dram_tensor("adj", [N, N], F32, kind="ExternalInput").ap()
    # idxpack[p, 0, t]=input_node, [1]=linenode, [2]=inputtext,
    # [3]=res (f32 bits), all in the (p, t) = index t*128+p layout
    idxpack = nc.dram_tensor("idxpack", [P, 4, NL // P], I32,
                             kind="ExternalInput").ap()
    tok_emb = nc.dram_tensor("tok_emb", [V, D - 1], F32,
                             kind="ExternalInput").ap()
    tok_emb1 = nc.dram_tensor("tok_emb1", [V, D], F32,
                              kind="ExternalInput").ap()
    # wpack[b, p, 0, :]=in_W[b, p]; [1+k]=ug_W[b, k*128+p]; [3+k]=rg_W;
    # [5+k]=ht_W  — one contiguous 3.5KB line per partition per block
    wpack = nc.dram_tensor("wpack", [NBLOCKS, P, 7, D], F32,
                           kind="ExternalInput").ap()
    # bpack[p, 0:5]=in_b.T, [5:10]=ug_b.T, [10:15]=rg_b.T, [15:20]=ht_b.T,
    # [20]=res2_W[p], [21]=res2_b (replicated)
    bpack = nc.dram_tensor("bpack", [P, 4 * NBLOCKS + 2], F32,
                           kind="ExternalInput").ap()

    xout = nc.dram_tensor("xout", [NL, D], F32, kind="ExternalOutput").ap()
    smout = nc.dram_tensor("smout", [1, NL], F32, kind="ExternalOutput").ap()
    lossout = nc.dram_tensor("lossout", [1, 1], F32,
                             kind="ExternalOutput").ap()

    with tile.TileContext(nc) as tc, ExitStack() as ctx:
        p_adjT = ctx.enter_context(tc.tile_pool(name="adjT", bufs=1))
        p_state = ctx.enter_context(tc.tile_pool(name="state", bufs=1))
        p_ch = ctx.enter_context(tc.tile_pool(name="ch", bufs=2))
        p_msg = ctx.enter_context(tc.tile_pool(name="msg", bufs=2))
        p_w = ctx.enter_context(tc.tile_pool(name="w", bufs=2))
        p_const = ctx.enter_context(tc.tile_pool(name="const", bufs=1))
        ps_tp = ctx.enter_context(
            tc.tile_pool(name="ps_tp", bufs=3, space="PSUM"))
        ps_mm = ctx.enter_context(
            tc.tile_pool(name="ps_mm", bufs=2, space="PSUM"))
        ps_g = ctx.enter_context(
            tc.tile_pool(name="ps_g", bufs=3, space="PSUM"))

        ident = p_const.tile([P, P], F32)
        make_identity(nc, ident[:])
        ident_r = p_const.tile([P, P], F32)
        nc.vector.tensor_copy(_r(ident_r[:]), ident[:])

        # persistent state, feature-major: hT[d, n]
        hT = p_state.tile([P, N], F32)
        # adjT_big[p, mt*N + n] = adj[n, mt*128 + p]
        adjT_big = p_adjT.tile([P, NT * N], F32)
        adjT_v = adjT_big.rearrange("p (m n) -> p m n", m=NT)

        # ---- biases / small constants (one packed DMA) ----
        NB4 = 4 * NBLOCKS
        bp = p_const.tile([P, NB4 + 2], F32)
        nc.sync.dma_start(bp[:], bpack[:, :])
        bias_in = bp[:, 0:NBLOCKS]
        bias_ug = bp[:, NBLOCKS:2 * NBLOCKS]
        bias_rg = bp[:, 2 * NBLOCKS:3 * NBLOCKS]
        bias_ht = bp[:, 3 * NBLOCKS:NB4]
        res2b = bp[0:1, NB4 + 1:NB4 + 2]
        res2w = p_const.tile([P, 1], F32)
        nc.vector.tensor_copy(_r(res2w[:]), bp[:, NB4:NB4 + 1])

        # gather indices / text / res, one packed DMA: [p, tensor, t]
        idx8 = p_const.tile([P, 4, NL // P], I32)
        nc.sync.dma_start(idx8[:], idxpack[:, :, :])
        nidx_t = idx8[:, 0, :]
        lidx_t = idx8[:, 1, :]
        text_t = idx8[:, 2, :]
        res8 = idx8[:, 3, :].bitcast(F32)

        # resmask in the [128, 8] (p, t) layout: element (p,t) = node t*128+p
        mask8 = p_const.tile([P, NL // P], I32)
        masked8 = p_const.tile([P, NL // P], F32)
        nc.vector.memset(masked8[:], -1e9)
        ones_row = p_const.tile([1, P], F32)
        nc.vector.memset(ones_row[:], 1.0)
        ones_col = p_const.tile([P, 1], F32)
        nc.vector.memset(ones_col[:], 1.0)
        # ---- adjacency load + on-chip transpose ----
        # full-row staging: 8KB contiguous per partition line keeps the
        # HBM DMA near peak rate (2KB lines measured at only ~107 GB/s)
        nc.vector.tensor_scalar(out=mask8[:], in0=nidx_t, scalar1=2,
                                scalar2=None, op0=mybir.AluOpType.is_equal)
        with tc.tile_pool(name="stage", bufs=2) as p_stage:
            for nb in range(NT):
                st = p_stage.tile([P, N], F32, tag="adj")
                eng = (nc.sync, nc.gpsimd, nc.scalar, nc.gpsimd)[nb % 4]
                eng.dma_start(st[:], adj[nb * P:(nb + 1) * P, :])
                for q in range(4):
                    mt0 = q * 4
                    pt = ps_tp.tile([P, CH], F32, tag="tp")
                    for j in range(4):
                        nc.tensor.transpose(
                            pt[:, j * P:(j + 1) * P],
                            st[:, (q * 4 + j) * P:(q * 4 + j + 1) * P],
                            ident[:])
                    # strided scatter into adjT_big: 4 m-tiles, n-block nb
                    nc.vector.tensor_copy(
                        _r(adjT_v[:, mt0:mt0 + 4, nb * P:(nb + 1) * P]),
                        pt[:].rearrange("p (m n) -> p m n", m=4))

        # warm the ACT Exp/Ln tables so the softmax tail doesn't pay
        # the two ~1.3us ACT_TABLE_LOADs serially at the end
        warm = p_const.tile([1, 1], F32)
        nc.scalar.activation(warm[:], res2b[:],
                             mybir.ActivationFunctionType.Exp)
        nc.scalar.activation(warm[:], warm[:],
                             mybir.ActivationFunctionType.Ln)


        # ---- embeddings -> hT (initial x, feature-major) ----
        # node embedding tile = [tok_emb row, text scalar] (128 features),
        # assembled node-major then PE-transposed into hT
        for g in range(NL // P // 4):
            pt = ps_tp.tile([P, CH], F32, tag="tp")
            for j in range(4):
                t = g * 4 + j
                ge = p_ch.tile([P, D], F32, tag="z")
                nc.gpsimd.indirect_dma_start(
                    out=ge[:, 0:D - 1], out_offset=None, in_=tok_emb[:, :],
                    in_offset=IndirectOffsetOnAxis(ap=nidx_t[:, t:t + 1],
                                                   axis=0))
                nc.vector.tensor_copy(ge[:, D - 1:D], text_t[:, t:t + 1])
                nc.tensor.transpose(pt[:, j * P:(j + 1) * P], ge[:],
                                    ident[:])
            nc.vector.tensor_copy(_r(hT[:, g * CH:(g + 1) * CH]), pt[:])

        for g in range(LL // P // 4):
            pt = ps_tp.tile([P, CH], F32, tag="tp")
            for j in range(4):
                t = g * 4 + j
                ge1 = p_ch.tile([P, D], F32, tag="z")
                nc.gpsimd.indirect_dma_start(
                    out=ge1[:], out_offset=None, in_=tok_emb1[:, :],
                    in_offset=IndirectOffsetOnAxis(ap=lidx_t[:, t:t + 1],
                                                   axis=0))
                nc.tensor.transpose(pt[:, j * P:(j + 1) * P], ge1[:],
                                    ident[:])
            nc.vector.tensor_copy(_r(hT[:, NL + g * CH:NL + (g + 1) * CH]),
                                  pt[:])

        # ---- GGNN blocks ----
        # One software pipeline across chunks, steps, and blocks:
        #  - h_nat transpose groups for the next step are emitted after this
        #    step's chunk update (lag 2), the last group carried into the
        #    next step's first message accumulation;
        #  - the next block's input transform rides the same "post" slots of
        #    the previous block's final step;
        #  - the network's final step only computes the node half (chunks
        #    0,1) and emits logits + x-output inline.
        def new_hnat():
            h_nat = p_state.tile([P, N], F32, tag="h_nat", bufs=2,
                                 name="h_nat")
            return h_nat

        def make_tp(h_dst, c):
            def emit():
                pt = ps_tp.tile([P, CH], F32, tag="tp", name="pt_tp")
                for j in range(4):
                    nb = c * 4 + j
                    nc.tensor.transpose(_r(pt[:, j * P:(j + 1) * P]),
                                        _r(hT[:, nb * P:(nb + 1) * P]),
                                        _r(ident_r[:]))
                if COPIES_ON_ACT:
                    nc.scalar.copy(_r(h_dst[:, c * CH:(c + 1) * CH]), pt[:])
                else:
                    nc.vector.tensor_copy(_r(h_dst[:, c * CH:(c + 1) * CH]),
                                          pt[:])
            return emit

        def load_weights(blk):
            w = {}
            ws = p_w.tile([P, 7, D], F32, tag="ws", name="ws")
            nc.sync.dma_start(ws[:], wpack[blk])
            w["in"] = p_w.tile([P, D], F32, tag="w_in", name="w_in")
            nc.vector.tensor_copy(_r(w["in"][:]), ws[:, 0, :])
            for i, key in enumerate(("ug", "rg", "ht")):
                w[key] = p_w.tile([P, 2, D], F32, tag="w_" + key,
                                  name="w_" + key)
                nc.vector.tensor_copy(_r(w[key][:]),
                                      ws[:, 1 + 2 * i:3 + 2 * i, :])
            return w

        def make_transform(w_next, blk_next, h_dst, c):
            # h = x @ in_W + in_b for chunk c of the next block (phase 1:
            # the matmul; the DVE bias-add runs between), then the transpose
            # group feeding its first step (phase 2)
            tp = make_tp(h_dst, c)

            def phase_mm():
                cs = slice(c * CH, (c + 1) * CH)
                pm = ps_g.tile([P, CH], F32, tag="g", name="pm")
                nc.tensor.matmul(pm[:], _r(w_next["in"][:]), _r(hT[:, cs]),
                                 start=True, stop=True)
                nc.vector.tensor_scalar(
                    out=_r(hT[:, cs]), in0=pm[:],
                    scalar1=bias_in[:, blk_next:blk_next + 1], scalar2=None,
                    op0=mybir.AluOpType.add)
            return phase_mm, tp

        xout_v = xout.rearrange("(a p) d -> p a d", p=P)
        l8psum_box = []

        def make_final_post(c):
            # logits columns (l8[p, t] = logits[t*128+p], +res2b via a
            # rank-1 accumulate) + x-output transposes for node chunk c
            def emit():
                if not l8psum_box:
                    l8psum_box.append(
                        ps_mm.tile([P, NL // P], F32, tag="m", name="l8"))
                l8psum = l8psum_box[0]
                for j in range(4):
                    nb = c * 4 + j
                    nc.tensor.matmul(l8psum[:, nb:nb + 1],
                                     hT[:, nb * P:(nb + 1) * P],
                                     res2w[:], start=True, stop=False)
                    nc.tensor.matmul(l8psum[:, nb:nb + 1], ones_row[:],
                                     res2b[:], start=False, stop=True)
                pt = ps_tp.tile([P, CH], F32, tag="tp", name="pt_x")
                for j in range(4):
                    nb = c * 4 + j
                    nc.tensor.transpose(pt[:, j * P:(j + 1) * P],
                                        hT[:, nb * P:(nb + 1) * P],
                                        ident[:])
                xs = p_ch.tile([P, CH], F32, tag="z", name="xs")
                nc.vector.tensor_copy(xs[:], pt[:])
                nc.sync.dma_start(xout_v[:, c * 4:(c + 1) * 4, :],
                                  xs[:].rearrange("p (a d) -> p a d", a=4))
            return emit

        # block 0 input transform (reads the embedding state)
        w_cur = load_weights(0)
        carry_tp = None
        h_cur = new_hnat()
        tp_q = []
        for c in range(NCH):
            pm = ps_g.tile([P, CH], F32, tag="g")
            nc.tensor.matmul(pm[:], _r(w_cur["in"][:]),
                             _r(hT[:, c * CH:(c + 1) * CH]),
                             start=True, stop=True)
            nc.vector.tensor_scalar(
                out=_r(hT[:, c * CH:(c + 1) * CH]), in0=pm[:],
                scalar1=bias_in[:, 0:1], scalar2=None,
                op0=mybir.AluOpType.add)
            if c >= 2:
                tp_q.pop(0)()
            tp_q.append(make_tp(h_cur, c))
        tp_q.pop(0)()
        carry_tp = tp_q.pop(0)

        for blk in range(nblocks):
            w_next = load_weights(blk + 1) if blk + 1 < nblocks else None
            w_ug, w_rg, w_ht = w_cur["ug"], w_cur["rg"], w_cur["ht"]

            for step in range(nsteps):
                last_step = step == nsteps - 1
                final_net_step = last_step and blk == nblocks - 1
                h_next = None if last_step else new_hnat()
                if final_net_step:
                    h_post = None

                    def make_post(c):
                        return None, make_final_post(c)
                elif last_step:
                    h_post = new_hnat()

                    def make_post(c, _h=h_post, _w=w_next, _b=blk + 1):
                        return make_transform(_w, _b, _h, c)
                else:
                    h_post = h_next

                    def make_post(c, _h=h_next):
                        return None, make_tp(_h, c)

                n_chunks = 2 if final_net_step else NCH
                tails = []
                posts = []
                for c in range(n_chunks):
                    cs = slice(c * CH, (c + 1) * CH)
                    pmsg = ps_mm.tile([P, CH], F32, tag="m", name="pmsg")
                    pz = ps_g.tile([P, CH], F32, tag="g", name="pz")
                    pr = ps_g.tile([P, CH], F32, tag="g", name="pr")

                    for mt in range(8):
                        nc.tensor.matmul(
                            pmsg[:], _r(h_cur[:, mt * P:(mt + 1) * P]),
                            _r(adjT_v[:, mt, c * CH:(c + 1) * CH]),
                            start=(mt == 0), stop=False)
                    nc.tensor.matmul(pz[:], _r(w_ug[:, 0, :]), _r(hT[:, cs]),
                                     start=True, stop=False)
                    nc.tensor.matmul(pr[:], _r(w_rg[:, 0, :]), _r(hT[:, cs]),
                                     start=True, stop=False)
                    if tails:
                        tails[0][0]()  # t_{c-1} rh-half
                    for mt in range(8, NT):
                        if mt == 8 and c == 0 and carry_tp is not None:
                            carry_tp()
                            carry_tp = None
                        nc.tensor.matmul(
                            pmsg[:], _r(h_cur[:, mt * P:(mt + 1) * P]),
                            _r(adjT_v[:, mt, c * CH:(c + 1) * CH]),
                            start=False, stop=(mt == NT - 1))
                    msgs_c = p_msg.tile([P, CH], F32, tag="msg",
                                        name="msgs_c")
                    if COPIES_ON_ACT:
                        nc.scalar.copy(_r(msgs_c[:]), pmsg[:])
                    else:
                        nc.vector.tensor_copy(_r(msgs_c[:]), pmsg[:])

                    if tails:
                        tails.pop(0)[1]()  # t_{c-1} finish + combine
                    if posts and posts[0][0] is not None:
                        posts[0][0]()  # transform matmul of post(c-1)
                        posts[0] = (None, posts[0][1])
                    if c >= 2 and posts:
                        posts.pop(0)[1]()

                    nc.tensor.matmul(pz[:], _r(w_ug[:, 1, :]), _r(msgs_c[:]),
                                     start=False, stop=True)
                    z_sb = p_ch.tile([P, CH], F32, tag="z")
                    nc.scalar.activation(z_sb[:], pz[:],
                                         mybir.ActivationFunctionType.Sigmoid,
                                         bias=bias_ug[:, blk:blk + 1])
                    nc.tensor.matmul(pr[:], _r(w_rg[:, 1, :]), _r(msgs_c[:]),
                                     start=False, stop=True)
                    r_c = p_ch.tile([P, CH], F32, tag="r", bufs=1)
                    nc.scalar.activation(r_c[:], pr[:],
                                         mybir.ActivationFunctionType.Sigmoid,
                                         bias=bias_rg[:, blk:blk + 1])
                    rh_c = p_ch.tile([P, CH], F32, tag="rh")
                    nc.vector.tensor_mul(_r(rh_c[:]), r_c[:], hT[:, cs])

                    def make_tail(c, z_sb, msgs_c, rh_c):
                        cs = slice(c * CH, (c + 1) * CH)
                        pt2 = ps_g.tile([P, CH], F32, tag="g", name="pt2")

                        def phase_a():
                            nc.tensor.matmul(pt2[:], _r(w_ht[:, 0, :]),
                                             _r(rh_c[:]), start=True,
                                             stop=False)

                        def phase_b():
                            nc.tensor.matmul(pt2[:], _r(w_ht[:, 1, :]),
                                             _r(msgs_c[:]), start=False,
                                             stop=True)
                            t_sb = p_ch.tile([P, CH], F32, tag="t",
                                             name="t_sb")
                            nc.scalar.activation(
                                t_sb[:], pt2[:],
                                mybir.ActivationFunctionType.Tanh,
                                bias=bias_ht[:, blk:blk + 1])
                            d_c = p_ch.tile([P, CH], F32, tag="d", bufs=1,
                                            name="d_c")
                            nc.vector.tensor_sub(d_c[:], hT[:, cs], t_sb[:])
                            nc.vector.tensor_mul(d_c[:], z_sb[:], d_c[:])
                            nc.vector.tensor_add(_r(hT[:, cs]), d_c[:],
                                                 t_sb[:])
                        return phase_a, phase_b

                    tails.append(make_tail(c, z_sb, msgs_c, rh_c))
                    posts.append(make_post(c))

                # epilogue: one post first to cover the rh roundtrip,
                # then the last tail, then the rest (last one carried)
                def run_post(p):
                    if p[0] is not None:
                        p[0]()
                    p[1]()

                if posts:
                    run_post(posts.pop(0))
                pa, pb = tails.pop(0)
                pa()
                pb()
                if final_net_step:
                    for p in posts:
                        run_post(p)
                    posts = []
                elif posts:
                    while len(posts) > 1:
                        run_post(posts.pop(0))
                    pmm, ptp = posts.pop(0)
                    if pmm is not None:
                        pmm()

                    def carry_fn(_tp=ptp):
                        _tp()
                    carry_tp = carry_fn
                h_cur = h_next if not last_step else (
                    h_post if not final_net_step else None)

            w_cur = w_next

        # softmax + loss, partition-parallel on the [128, 8] layout
        T8 = NL // P
        l8psum = l8psum_box[0]
        nc.vector.copy_predicated(masked8[:], mask8[:], l8psum[:])

        mx1 = p_ch.tile([P, 1], F32, tag="d", bufs=1)
        nc.vector.reduce_max(mx1[:], masked8[:], axis=mybir.AxisListType.X)
        pmx = ps_g.tile([1, P], F32, tag="g")
        nc.tensor.transpose(pmx[:], mx1[:], ident[:])
        negmx = p_const.tile([1, 1], F32)
        nc.vector.reduce_max(negmx[:], pmx[:], axis=mybir.AxisListType.X,
                             negate=True)
        pbc = ps_g.tile([P, 1], F32, tag="g")
        nc.tensor.matmul(pbc[:], ones_row[:], negmx[:], start=True,
                         stop=True)
        negmx_bc = p_const.tile([P, 1], F32)
        nc.vector.tensor_copy(negmx_bc[:], pbc[:])
        ex8 = p_ch.tile([P, T8], F32, tag="d", bufs=1)
        nc.scalar.activation(ex8[:], masked8[:],
                             mybir.ActivationFunctionType.Exp,
                             bias=negmx_bc[:], scale=1.0)
        s1 = p_const.tile([P, 1], F32)
        nc.vector.reduce_sum(s1[:], ex8[:], axis=mybir.AxisListType.X)
        psum1 = ps_g.tile([1, 1], F32, tag="g")
        nc.tensor.matmul(psum1[:], s1[:], ones_col[:], start=True,
                         stop=True)
        rs = p_const.tile([1, 1], F32)
        nc.vector.tensor_copy(rs[:], psum1[:])
        nc.vector.reciprocal(rs[:], rs[:])
        prs = ps_g.tile([P, 1], F32, tag="g")
        nc.tensor.matmul(prs[:], ones_row[:], rs[:], start=True, stop=True)
        rs_bc = p_const.tile([P, 1], F32)
        nc.vector.tensor_copy(rs_bc[:], prs[:])
        smx8 = p_ch.tile([P, T8], F32, tag="rh")
        nc.vector.tensor_scalar_mul(smx8[:], ex8[:], rs_bc[:])
        nc.sync.dma_start(
            smout.rearrange("a (t p) -> p (a t)", p=P), smx8[:])

        # loss = -(log(clip(softmax, 1e-10, 1)) * res).sum()
        cl8 = p_ch.tile([P, T8], F32, tag="r", bufs=1)
        nc.vector.tensor_scalar(out=cl8[:], in0=smx8[:], scalar1=1e-10,
                                scalar2=1.0, op0=mybir.AluOpType.max,
                                op1=mybir.AluOpType.min)
        ln8 = p_ch.tile([P, T8], F32, tag="z")
        nc.scalar.activation(ln8[:], cl8[:],
                             mybir.ActivationFunctionType.Ln)
        pr8 = p_ch.tile([P, T8], F32, tag="t")
        nc.vector.tensor_mul(pr8[:], ln8[:], res8[:])
        ps1 = p_const.tile([P, 1], F32)
        nc.vector.reduce_sum(ps1[:], pr8[:], axis=mybir.AxisListType.X)
        pls = ps_g.tile([1, 1], F32, tag="g")
        nc.tensor.matmul(pls[:], ps1[:], ones_col[:], start=True,
                         stop=True)
        ls = p_const.tile([1, 1], F32)
        nc.vector.tensor_scalar_mul(ls[:], pls[:], -1.0)
        nc.sync.dma_start(lossout[:, :], ls[:])

    nc.compile()
    return nc


_NC = None
LAST_RESULT = None


def _get_nc():
    global _NC
    if _NC is None:
        _NC = build_nc()
    return _NC


def make_in_maps(inputs):
    """Host-side marshalling: per-core slices + packed weight/bias/index
    tensors laid out for contiguous partition-major DMA."""
    adj = np.asarray(inputs["inputad"], np.float32)
    nidx = np.asarray(inputs["input_node"]).astype(np.int32)
    lidx = np.asarray(inputs["linenode"]).astype(np.int32)
    text = np.asarray(inputs["inputtext"]).astype(np.int32)
    res = np.asarray(inputs["res"]).astype(np.float32)

    in_W = np.asarray(inputs["in_W"], np.float32)
    ug_W = np.asarray(inputs["ug_W"], np.float32)
    rg_W = np.asarray(inputs["rg_W"], np.float32)
    ht_W = np.asarray(inputs["ht_W"], np.float32)
    wpack = np.empty([NBLOCKS, P, 7, D], np.float32)
    wpack[:, :, 0] = in_W
    wpack[:, :, 1] = ug_W[:, 0:P]
    wpack[:, :, 2] = ug_W[:, P:2 * P]
    wpack[:, :, 3] = rg_W[:, 0:P]
    wpack[:, :, 4] = rg_W[:, P:2 * P]
    wpack[:, :, 5] = ht_W[:, 0:P]
    wpack[:, :, 6] = ht_W[:, P:2 * P]
    wpack = np.ascontiguousarray(wpack)

    bpack = np.empty([P, 4 * NBLOCKS + 2], np.float32)
    bpack[:, 0:NBLOCKS] = np.asarray(inputs["in_b"], np.float32).T
    bpack[:, NBLOCKS:2 * NBLOCKS] = np.asarray(inputs["ug_b"], np.float32).T
    bpack[:, 2 * NBLOCKS:3 * NBLOCKS] = np.asarray(inputs["rg_b"],
                                                   np.float32).T
    bpack[:, 3 * NBLOCKS:4 * NBLOCKS] = np.asarray(inputs["ht_b"],
                                                   np.float32).T
    bpack[:, 4 * NBLOCKS] = np.asarray(inputs["res2_W"], np.float32)[:, 0]
    bpack[:, 4 * NBLOCKS + 1] = float(np.asarray(inputs["res2_b"],
                                                 np.float32).ravel()[0])
    bpack = np.ascontiguousarray(bpack)

    shared = {
        "tok_emb": np.ascontiguousarray(
            np.asarray(inputs["tok_emb"], np.float32)),
        "tok_emb1": np.ascontiguousarray(
            np.asarray(inputs["tok_emb1"], np.float32)),
        "wpack": wpack,
        "bpack": bpack,
    }
    in_maps = []
    for b in range(N_CORES):
        idxpack = np.empty([P, 4, NL // P], np.int32)
        idxpack[:, 0] = nidx[b].reshape(NL // P, P).T
        idxpack[:, 1] = lidx[b].reshape(LL // P, P).T
        idxpack[:, 2] = text[b].reshape(NL // P, P).T
        idxpack[:, 3] = res[b].reshape(NL // P, P).T.view(np.int32)
        in_maps.append({
            "adj": np.ascontiguousarray(adj[b]),
            "idxpack": np.ascontiguousarray(idxpack),
            **shared,
        })
    return in_maps


def kernel(**inputs):
    nc = _get_nc()
    in_maps = make_in_maps(inputs)

    from concourse.bass_utils import run_bass_kernel_spmd
    global LAST_RESULT
    LAST_RESULT = run_bass_kernel_spmd(nc, in_maps,
                                       core_ids=list(range(N_CORES)))

    loss = np.zeros([B], np.float32)
    softmax = np.zeros([B, NL], np.float32)
    x = np.zeros([B, NL, D], np.float32)
    for b in range(N_CORES):
        r = LAST_RESULT.results[b]
        loss[b] = r["lossout"][0, 0]
        softmax[b] = r["smout"][0]
        x[b] = r["xout"]
    return loss, softmax, x


# revision 37
# speedup vs baseline: 1.0393x; 1.0393x over previous
"""GGNN message-passing encoder on 8 Trainium2 NeuronCores.

Data-parallel over batch B=8: core b processes batch element b end-to-end
(its own [N,N] adjacency slice; small GGNN weights replicated), no
collectives. The whole working set (adjT 16.8 MB + state + weights) lives
in SBUF, so the adjacency is read from HBM exactly once and reused for all
NBLOCKS*NSTEPS message-passing matmuls.

Kernel layout: the state is kept feature-major (hT [D=128 part, N=2048
free]) so every matmul has a 512-wide moving operand and runs at full PE
rate in float32r. The adjacency is transposed on-chip (PE transposes,
grouped 4-per-PSUM-bank) into adjT[m, n] once at load time.
"""

import sys

sys.path.insert(0, "/opt/trn_rl_repo")

from contextlib import ExitStack

import numpy as np

import concourse.bass as bass
import concourse.mybir as mybir
import concourse.tile as tile
from concourse import bacc
from concourse.bass import IndirectOffsetOnAxis
from concourse.masks import make_identity

P = 128
B = 8
NL = 1024
LL = 1024
N = NL + LL          # 2048 nodes
D = 128
V = 50000
NBLOCKS = 5
NSTEPS = 3
NT = N // P          # 16 node tiles
CH = 512             # n-chunk (PSUM bank width in fp32)
NCH = N // CH        # 4 chunks
F32 = mybir.dt.float32
F32R = mybir.dt.float32r
I32 = mybir.dt.int32

N_CORES = 8
COPIES_ON_ACT = True


def _r(ap):
    """View an fp32 AP as float32r for full-rate PE matmuls."""
    return ap.bitcast(F32R)


def build_nc(nblocks=NBLOCKS, nsteps=NSTEPS):
    nc = bacc.Bacc("TRN2", target_bir_lowering=False, debug=False,
                   num_devices=N_CORES)

    # ---- per-core DRAM tensors (each core gets its own batch slice) ----
    adj = nc.dram_tensor("adj", [N, N], F32, kind="ExternalInput").ap()
    # idxpack[p, 0, t]=input_node, [1]=linenode, [2]=inputtext,
    # [3]=res (f32 bits), all in the (p, t) = index t*128+p layout
    idxpack = nc.dram_tensor("idxpack", [P, 4, NL // P], I32,
                             kind="ExternalInput").ap()
    tok_emb = nc.dram_tensor("tok_emb", [V, D - 1], F32,
                             kind="ExternalInput").ap()
    tok_emb1 = nc.dram_tensor("tok_emb1", [V, D], F32,
                              kind="ExternalInput").ap()
    # wpack[b, p, 0, :]=in_W[b, p]; [1+k]=ug_W[b, k*128+p]; [3+k]=rg_W;
    # [5+k]=ht_W  — one contiguous 3.5KB line per partition per block
    wpack = nc.dram_tensor("wpack", [NBLOCKS, P, 7, D], F32,
                           kind="ExternalInput").ap()
    # bpack[p, 0:5]=in_b.T, [5:10]=ug_b.T, [10:15]=rg_b.T, [15:20]=ht_b.T,
    # [20]=res2_W[p], [21]=res2_b (replicated)
    bpack = nc.dram_tensor("bpack", [P, 4 * NBLOCKS + 2], F32,
                           kind="ExternalInput").ap()

    xout = nc.dram_tensor("xout", [NL, D], F32, kind="ExternalOutput").ap()
    smout = nc.dram_tensor("smout", [1, NL], F32, kind="ExternalOutput").ap()
    lossout = nc.dram_tensor("lossout", [1, 1], F32,
                             kind="ExternalOutput").ap()

    with tile.TileContext(nc) as tc, ExitStack() as ctx:
        p_adjT = ctx.enter_context(tc.tile_pool(name="adjT", bufs=1))
        p_state = ctx.enter_context(tc.tile_pool(name="state", bufs=1))
        p_ch = ctx.enter_context(tc.tile_pool(name="ch", bufs=2))
        p_msg = ctx.enter_context(tc.tile_pool(name="msg", bufs=2))
        p_w = ctx.enter_context(tc.tile_pool(name="w", bufs=2))
        p_const = ctx.enter_context(tc.tile_pool(name="const", bufs=1))
        ps_tp = ctx.enter_context(
            tc.tile_pool(name="ps_tp", bufs=3, space="PSUM"))
        ps_mm = ctx.enter_context(
            tc.tile_pool(name="ps_mm", bufs=2, space="PSUM"))
        ps_g = ctx.enter_context(
            tc.tile_pool(name="ps_g", bufs=3, space="PSUM"))

        ident = p_const.tile([P, P], F32)
        make_identity(nc, ident[:])
        ident_r = p_const.tile([P, P], F32)
        nc.vector.tensor_copy(_r(ident_r[:]), ident[:])

        # persistent state, feature-major: hT[d, n]
        hT = p_state.tile([P, N], F32)
        # adjT_big[p, mt*N + n] = adj[n, mt*128 + p]
        adjT_big = p_adjT.tile([P, NT * N], F32)
        adjT_v = adjT_big.rearrange("p (m n) -> p m n", m=NT)

        # ---- biases / small constants (one packed DMA) ----
        NB4 = 4 * NBLOCKS
        bp = p_const.tile([P, NB4 + 2], F32)
        nc.sync.dma_start(bp[:], bpack[:, :])
        bias_in = bp[:, 0:NBLOCKS]
        bias_ug = bp[:, NBLOCKS:2 * NBLOCKS]
        bias_rg = bp[:, 2 * NBLOCKS:3 * NBLOCKS]
        bias_ht = bp[:, 3 * NBLOCKS:NB4]
        res2b = bp[0:1, NB4 + 1:NB4 + 2]
        res2w = p_const.tile([P, 1], F32)
        nc.vector.tensor_copy(_r(res2w[:]), bp[:, NB4:NB4 + 1])

        # gather indices / text / res, one packed DMA: [p, tensor, t]
        idx8 = p_const.tile([P, 4, NL // P], I32)
        nc.sync.dma_start(idx8[:], idxpack[:, :, :])
        nidx_t = idx8[:, 0, :]
        lidx_t = idx8[:, 1, :]
        text_t = idx8[:, 2, :]
        res8 = idx8[:, 3, :].bitcast(F32)

        # resmask in the [128, 8] (p, t) layout: element (p,t) = node t*128+p
        mask8 = p_const.tile([P, NL // P], I32)
        masked8 = p_const.tile([P, NL // P], F32)
        nc.vector.memset(masked8[:], -1e9)
        ones_row = p_const.tile([1, P], F32)
        nc.vector.memset(ones_row[:], 1.0)
        ones_col = p_const.tile([P, 1], F32)
        nc.vector.memset(ones_col[:], 1.0)
        # ---- adjacency load + on-chip transpose ----
        # full-row staging: 8KB contiguous per partition line keeps the
        # HBM DMA near peak rate (2KB lines measured at only ~107 GB/s)
        nc.vector.tensor_scalar(out=mask8[:], in0=nidx_t, scalar1=2,
                                scalar2=None, op0=mybir.AluOpType.is_equal)
        with tc.tile_pool(name="stage", bufs=3) as p_stage:
            for nb in range(NT):
                st = p_stage.tile([P, N], F32, tag="adj")
                eng = (nc.sync, nc.scalar, nc.gpsimd, nc.sync,
                       nc.scalar, nc.gpsimd, nc.sync, nc.scalar)[nb % 8]
                eng.dma_start(st[:], adj[nb * P:(nb + 1) * P, :])
                for q in range(4):
                    mt0 = q * 4
                    pt = ps_tp.tile([P, CH], F32, tag="tp")
                    for j in range(4):
                        nc.tensor.transpose(
                            pt[:, j * P:(j + 1) * P],
                            st[:, (q * 4 + j) * P:(q * 4 + j + 1) * P],
                            ident[:])
                    # strided scatter into adjT_big: 4 m-tiles, n-block nb
                    nc.vector.tensor_copy(
                        _r(adjT_v[:, mt0:mt0 + 4, nb * P:(nb + 1) * P]),
                        pt[:].rearrange("p (m n) -> p m n", m=4))

        # warm the ACT Exp/Ln tables so the softmax tail doesn't pay
        # the two ~1.3us ACT_TABLE_LOADs serially at the end
        warm = p_const.tile([1, 1], F32)
        nc.scalar.activation(warm[:], res2b[:],
                             mybir.ActivationFunctionType.Exp)
        nc.scalar.activation(warm[:], warm[:],
                             mybir.ActivationFunctionType.Ln)


        # ---- embeddings -> hT (initial x, feature-major) ----
        # node embedding tile = [tok_emb row, text scalar] (128 features),
        # assembled node-major then PE-transposed into hT
        for g in range(NL // P // 4):
            pt = ps_tp.tile([P, CH], F32, tag="tp")
            for j in range(4):
                t = g * 4 + j
                ge = p_ch.tile([P, D], F32, tag="z")
                nc.gpsimd.indirect_dma_start(
                    out=ge[:, 0:D - 1], out_offset=None, in_=tok_emb[:, :],
                    in_offset=IndirectOffsetOnAxis(ap=nidx_t[:, t:t + 1],
                                                   axis=0))
                nc.vector.tensor_copy(ge[:, D - 1:D], text_t[:, t:t + 1])
                nc.tensor.transpose(pt[:, j * P:(j + 1) * P], ge[:],
                                    ident[:])
            nc.vector.tensor_copy(_r(hT[:, g * CH:(g + 1) * CH]), pt[:])

        for g in range(LL // P // 4):
            pt = ps_tp.tile([P, CH], F32, tag="tp")
            for j in range(4):
                t = g * 4 + j
                ge1 = p_ch.tile([P, D], F32, tag="z")
                nc.gpsimd.indirect_dma_start(
                    out=ge1[:], out_offset=None, in_=tok_emb1[:, :],
                    in_offset=IndirectOffsetOnAxis(ap=lidx_t[:, t:t + 1],
                                                   axis=0))
                nc.tensor.transpose(pt[:, j * P:(j + 1) * P], ge1[:],
                                    ident[:])
            nc.vector.tensor_copy(_r(hT[:, NL + g * CH:NL + (g + 1) * CH]),
                                  pt[:])

        # ---- GGNN blocks ----
        # One software pipeline across chunks, steps, and blocks:
        #  - h_nat transpose groups for the next step are emitted after this
        #    step's chunk update (lag 2), the last group carried into the
        #    next step's first message accumulation;
        #  - the next block's input transform rides the same "post" slots of
        #    the previous block's final step;
        #  - the network's final step only computes the node half (chunks
        #    0,1) and emits logits + x-output inline.
        def new_hnat():
            h_nat = p_state.tile([P, N], F32, tag="h_nat", bufs=2,
                                 name="h_nat")
            return h_nat

        def make_tp(h_dst, c):
            def emit():
                pt = ps_tp.tile([P, CH], F32, tag="tp", name="pt_tp")
                for j in range(4):
                    nb = c * 4 + j
                    nc.tensor.transpose(_r(pt[:, j * P:(j + 1) * P]),
                                        _r(hT[:, nb * P:(nb + 1) * P]),
                                        _r(ident_r[:]))
                if COPIES_ON_ACT:
                    nc.scalar.copy(_r(h_dst[:, c * CH:(c + 1) * CH]), pt[:])
                else:
                    nc.vector.tensor_copy(_r(h_dst[:, c * CH:(c + 1) * CH]),
                                          pt[:])
            return emit

        def load_weights(blk):
            w = {}
            ws = p_w.tile([P, 7, D], F32, tag="ws", bufs=1, name="ws")
            nc.sync.dma_start(ws[:], wpack[blk])
            w["in"] = p_w.tile([P, D], F32, tag="w_in", name="w_in")
            nc.vector.tensor_copy(_r(w["in"][:]), ws[:, 0, :])
            for i, key in enumerate(("ug", "rg", "ht")):
                w[key] = p_w.tile([P, 2, D], F32, tag="w_" + key,
                                  name="w_" + key)
                nc.vector.tensor_copy(_r(w[key][:]),
                                      ws[:, 1 + 2 * i:3 + 2 * i, :])
            return w

        def make_transform(w_next, blk_next, h_dst, c):
            # h = x @ in_W + in_b for chunk c of the next block (phase 1:
            # the matmul; the DVE bias-add runs between), then the transpose
            # group feeding its first step (phase 2)
            tp = make_tp(h_dst, c)

            def phase_mm():
                cs = slice(c * CH, (c + 1) * CH)
                pm = ps_g.tile([P, CH], F32, tag="g", name="pm")
                nc.tensor.matmul(pm[:], _r(w_next["in"][:]), _r(hT[:, cs]),
                                 start=True, stop=True)
                nc.vector.tensor_scalar(
                    out=_r(hT[:, cs]), in0=pm[:],
                    scalar1=bias_in[:, blk_next:blk_next + 1], scalar2=None,
                    op0=mybir.AluOpType.add)
            return phase_mm, tp

        xout_v = xout.rearrange("(a p) d -> p a d", p=P)
        l8psum_box = []

        def make_final_post(c):
            # logits columns (l8[p, t] = logits[t*128+p], +res2b via a
            # rank-1 accumulate) + x-output transposes for node chunk c
            def emit():
                if not l8psum_box:
                    l8psum_box.append(
                        ps_mm.tile([P, NL // P], F32, tag="m", name="l8"))
                l8psum = l8psum_box[0]
                for j in range(4):
                    nb = c * 4 + j
                    nc.tensor.matmul(l8psum[:, nb:nb + 1],
                                     hT[:, nb * P:(nb + 1) * P],
                                     res2w[:], start=True, stop=False)
                    nc.tensor.matmul(l8psum[:, nb:nb + 1], ones_row[:],
                                     res2b[:], start=False, stop=True)
                pt = ps_tp.tile([P, CH], F32, tag="tp", name="pt_x")
                for j in range(4):
                    nb = c * 4 + j
                    nc.tensor.transpose(pt[:, j * P:(j + 1) * P],
                                        hT[:, nb * P:(nb + 1) * P],
                                        ident[:])
                xs = p_ch.tile([P, CH], F32, tag="z", name="xs")
                nc.vector.tensor_copy(xs[:], pt[:])
                nc.sync.dma_start(xout_v[:, c * 4:(c + 1) * 4, :],
                                  xs[:].rearrange("p (a d) -> p a d", a=4))
            return emit

        # block 0 input transform (reads the embedding state)
        w_cur = load_weights(0)
        carry_tp = None
        h_cur = new_hnat()
        tp_q = []
        for c in range(NCH):
            pm = ps_g.tile([P, CH], F32, tag="g")
            nc.tensor.matmul(pm[:], _r(w_cur["in"][:]),
                             _r(hT[:, c * CH:(c + 1) * CH]),
                             start=True, stop=True)
            nc.vector.tensor_scalar(
                out=_r(hT[:, c * CH:(c + 1) * CH]), in0=pm[:],
                scalar1=bias_in[:, 0:1], scalar2=None,
                op0=mybir.AluOpType.add)
            if c >= 2:
                tp_q.pop(0)()
            tp_q.append(make_tp(h_cur, c))
        tp_q.pop(0)()
        carry_tp = tp_q.pop(0)

        for blk in range(nblocks):
            w_next = load_weights(blk + 1) if blk + 1 < nblocks else None
            w_ug, w_rg, w_ht = w_cur["ug"], w_cur["rg"], w_cur["ht"]

            for step in range(nsteps):
                last_step = step == nsteps - 1
                final_net_step = last_step and blk == nblocks - 1
                h_next = None if last_step else new_hnat()
                if final_net_step:
                    h_post = None

                    def make_post(c):
                        return None, make_final_post(c)
                elif last_step:
                    h_post = new_hnat()

                    def make_post(c, _h=h_post, _w=w_next, _b=blk + 1):
                        return make_transform(_w, _b, _h, c)
                else:
                    h_post = h_next

                    def make_post(c, _h=h_next):
                        return None, make_tp(_h, c)

                n_chunks = 2 if final_net_step else NCH
                tails = []
                posts = []
                for c in range(n_chunks):
                    cs = slice(c * CH, (c + 1) * CH)
                    pmsg = ps_mm.tile([P, CH], F32, tag="m", name="pmsg")
                    pz = ps_g.tile([P, CH], F32, tag="g", name="pz")
                    pr = ps_g.tile([P, CH], F32, tag="g", name="pr")

                    for mt in range(8):
                        nc.tensor.matmul(
                            pmsg[:], _r(h_cur[:, mt * P:(mt + 1) * P]),
                            _r(adjT_v[:, mt, c * CH:(c + 1) * CH]),
                            start=(mt == 0), stop=False)
                    nc.tensor.matmul(pz[:], _r(w_ug[:, 0, :]), _r(hT[:, cs]),
                                     start=True, stop=False)
                    nc.tensor.matmul(pr[:], _r(w_rg[:, 0, :]), _r(hT[:, cs]),
                                     start=True, stop=False)
                    if tails:
                        tails[0][0]()  # t_{c-1} rh-half
                    for mt in range(8, NT):
                        if mt == 8 and c == 0 and carry_tp is not None:
                            carry_tp()
                            carry_tp = None
                        nc.tensor.matmul(
                            pmsg[:], _r(h_cur[:, mt * P:(mt + 1) * P]),
                            _r(adjT_v[:, mt, c * CH:(c + 1) * CH]),
                            start=False, stop=(mt == NT - 1))
                    msgs_c = p_msg.tile([P, CH], F32, tag="msg",
                                        name="msgs_c")
                    if COPIES_ON_ACT:
                        nc.scalar.copy(_r(msgs_c[:]), pmsg[:])
                    else:
                        nc.vector.tensor_copy(_r(msgs_c[:]), pmsg[:])

                    if tails:
                        tails.pop(0)[1]()  # t_{c-1} finish + combine
                    if posts and posts[0][0] is not None:
                        posts[0][0]()  # transform matmul of post(c-1)
                        posts[0] = (None, posts[0][1])
                    if c >= 2 and posts:
                        posts.pop(0)[1]()

                    nc.tensor.matmul(pz[:], _r(w_ug[:, 1, :]), _r(msgs_c[:]),
                                     start=False, stop=True)
                    z_sb = p_ch.tile([P, CH], F32, tag="z")
                    nc.scalar.activation(z_sb[:], pz[:],
                                         mybir.ActivationFunctionType.Sigmoid,
                                         bias=bias_ug[:, blk:blk + 1])
                    nc.tensor.matmul(pr[:], _r(w_rg[:, 1, :]), _r(msgs_c[:]),
                                     start=False, stop=True)
                    r_c = p_ch.tile([P, CH], F32, tag="r", bufs=1)
                    nc.scalar.activation(r_c[:], pr[:],
                                         mybir.ActivationFunctionType.Sigmoid,
                                         bias=bias_rg[:, blk:blk + 1])
                    rh_c = p_ch.tile([P, CH], F32, tag="rh")
                    nc.vector.tensor_mul(_r(rh_c[:]), r_c[:], hT[:, cs])

                    def make_tail(c, z_sb, msgs_c, rh_c):
                        cs = slice(c * CH, (c + 1) * CH)
                        pt2 = ps_g.tile([P, CH], F32, tag="g", name="pt2")

                        def phase_a():
                            nc.tensor.matmul(pt2[:], _r(w_ht[:, 0, :]),
                                             _r(rh_c[:]), start=True,
                                             stop=False)

                        def phase_b():
                            nc.tensor.matmul(pt2[:], _r(w_ht[:, 1, :]),
                                             _r(msgs_c[:]), start=False,
                                             stop=True)
                            t_sb = p_ch.tile([P, CH], F32, tag="t", bufs=1,
                                             name="t_sb")
                            nc.scalar.activation(
                                t_sb[:], pt2[:],
                                mybir.ActivationFunctionType.Tanh,
                                bias=bias_ht[:, blk:blk + 1])
                            d_c = p_ch.tile([P, CH], F32, tag="d", bufs=1,
                                            name="d_c")
                            nc.vector.tensor_sub(d_c[:], hT[:, cs], t_sb[:])
                            nc.vector.tensor_mul(d_c[:], z_sb[:], d_c[:])
                            nc.vector.tensor_add(_r(hT[:, cs]), d_c[:],
                                                 t_sb[:])
                        return phase_a, phase_b

                    tails.append(make_tail(c, z_sb, msgs_c, rh_c))
                    posts.append(make_post(c))

                # epilogue: one post first to cover the rh roundtrip,
                # then the last tail, then the rest (last one carried)
                def run_post(p):
                    if p[0] is not None:
                        p[0]()
                    p[1]()

                if posts:
                    run_post(posts.pop(0))
                pa, pb = tails.pop(0)
                pa()
                pb()
                if final_net_step:
                    for p in posts:
                        run_post(p)
                    posts = []
                elif posts:
                    while len(posts) > 1:
                        run_post(posts.pop(0))
                    pmm, ptp = posts.pop(0)
                    if pmm is not None:
                        pmm()

                    def carry_fn(_tp=ptp):
                        _tp()
                    carry_tp = carry_fn
                h_cur = h_next if not last_step else (
                    h_post if not final_net_step else None)

            w_cur = w_next

        # softmax + loss, partition-parallel on the [128, 8] layout
        T8 = NL // P
        l8psum = l8psum_box[0]
        nc.vector.copy_predicated(masked8[:], mask8[:], l8psum[:])

        mx1 = p_ch.tile([P, 1], F32, tag="d", bufs=1)
        nc.vector.reduce_max(mx1[:], masked8[:], axis=mybir.AxisListType.X)
        pmx = ps_g.tile([1, P], F32, tag="g")
        nc.tensor.transpose(pmx[:], mx1[:], ident[:])
        negmx = p_const.tile([1, 1], F32)
        nc.vector.reduce_max(negmx[:], pmx[:], axis=mybir.AxisListType.X,
                             negate=True)
        pbc = ps_g.tile([P, 1], F32, tag="g")
        nc.tensor.matmul(pbc[:], ones_row[:], negmx[:], start=True,
                         stop=True)
        negmx_bc = p_const.tile([P, 1], F32)
        nc.vector.tensor_copy(negmx_bc[:], pbc[:])
        ex8 = p_ch.tile([P, T8], F32, tag="d", bufs=1)
        nc.scalar.activation(ex8[:], masked8[:],
                             mybir.ActivationFunctionType.Exp,
                             bias=negmx_bc[:], scale=1.0)
        s1 = p_const.tile([P, 1], F32)
        nc.vector.reduce_sum(s1[:], ex8[:], axis=mybir.AxisListType.X)
        psum1 = ps_g.tile([1, 1], F32, tag="g")
        nc.tensor.matmul(psum1[:], s1[:], ones_col[:], start=True,
                         stop=True)
        rs = p_const.tile([1, 1], F32)
        nc.vector.tensor_copy(rs[:], psum1[:])
        nc.vector.reciprocal(rs[:], rs[:])
        prs = ps_g.tile([P, 1], F32, tag="g")
        nc.tensor.matmul(prs[:], ones_row[:], rs[:], start=True, stop=True)
        rs_bc = p_const.tile([P, 1], F32)
        nc.vector.tensor_copy(rs_bc[:], prs[:])
        smx8 = p_ch.tile([P, T8], F32, tag="rh")
        nc.vector.tensor_scalar_mul(smx8[:], ex8[:], rs_bc[:])
        nc.sync.dma_start(
            smout.rearrange("a (t p) -> p (a t)", p=P), smx8[:])

        # loss = -(log(clip(softmax, 1e-10, 1)) * res).sum()
        cl8 = p_ch.tile([P, T8], F32, tag="r", bufs=1)
        nc.vector.tensor_scalar(out=cl8[:], in0=smx8[:], scalar1=1e-10,
                                scalar2=1.0, op0=mybir.AluOpType.max,
                                op1=mybir.AluOpType.min)
        ln8 = p_ch.tile([P, T8], F32, tag="z")
        nc.scalar.activation(ln8[:], cl8[:],
                             mybir.ActivationFunctionType.Ln)
        pr8 = p_ch.tile([P, T8], F32, tag="t", bufs=1)
        nc.vector.tensor_mul(pr8[:], ln8[:], res8[:])
        ps1 = p_const.tile([P, 1], F32)
        nc.vector.reduce_sum(ps1[:], pr8[:], axis=mybir.AxisListType.X)
        pls = ps_g.tile([1, 1], F32, tag="g")
        nc.tensor.matmul(pls[:], ps1[:], ones_col[:], start=True,
                         stop=True)
        ls = p_const.tile([1, 1], F32)
        nc.vector.tensor_scalar_mul(ls[:], pls[:], -1.0)
        nc.sync.dma_start(lossout[:, :], ls[:])

    nc.compile()
    return nc


_NC = None
LAST_RESULT = None


def _get_nc():
    global _NC
    if _NC is None:
        _NC = build_nc()
    return _NC


def make_in_maps(inputs):
    """Host-side marshalling: per-core slices + packed weight/bias/index
    tensors laid out for contiguous partition-major DMA."""
    adj = np.asarray(inputs["inputad"], np.float32)
    nidx = np.asarray(inputs["input_node"]).astype(np.int32)
    lidx = np.asarray(inputs["linenode"]).astype(np.int32)
    text = np.asarray(inputs["inputtext"]).astype(np.int32)
    res = np.asarray(inputs["res"]).astype(np.float32)

    in_W = np.asarray(inputs["in_W"], np.float32)
    ug_W = np.asarray(inputs["ug_W"], np.float32)
    rg_W = np.asarray(inputs["rg_W"], np.float32)
    ht_W = np.asarray(inputs["ht_W"], np.float32)
    wpack = np.empty([NBLOCKS, P, 7, D], np.float32)
    wpack[:, :, 0] = in_W
    wpack[:, :, 1] = ug_W[:, 0:P]
    wpack[:, :, 2] = ug_W[:, P:2 * P]
    wpack[:, :, 3] = rg_W[:, 0:P]
    wpack[:, :, 4] = rg_W[:, P:2 * P]
    wpack[:, :, 5] = ht_W[:, 0:P]
    wpack[:, :, 6] = ht_W[:, P:2 * P]
    wpack = np.ascontiguousarray(wpack)

    bpack = np.empty([P, 4 * NBLOCKS + 2], np.float32)
    bpack[:, 0:NBLOCKS] = np.asarray(inputs["in_b"], np.float32).T
    bpack[:, NBLOCKS:2 * NBLOCKS] = np.asarray(inputs["ug_b"], np.float32).T
    bpack[:, 2 * NBLOCKS:3 * NBLOCKS] = np.asarray(inputs["rg_b"],
                                                   np.float32).T
    bpack[:, 3 * NBLOCKS:4 * NBLOCKS] = np.asarray(inputs["ht_b"],
                                                   np.float32).T
    bpack[:, 4 * NBLOCKS] = np.asarray(inputs["res2_W"], np.float32)[:, 0]
    bpack[:, 4 * NBLOCKS + 1] = float(np.asarray(inputs["res2_b"],
                                                 np.float32).ravel()[0])
    bpack = np.ascontiguousarray(bpack)

    shared = {
        "tok_emb": np.ascontiguousarray(
            np.asarray(inputs["tok_emb"], np.float32)),
        "tok_emb1": np.ascontiguousarray(
            np.asarray(inputs["tok_emb1"], np.float32)),
        "wpack": wpack,
        "bpack": bpack,
    }
    in_maps = []
    for b in range(N_CORES):
        idxpack = np.empty([P, 4, NL // P], np.int32)
        idxpack[:, 0] = nidx[b].reshape(NL // P, P).T
        idxpack[:, 1] = lidx[b].reshape(LL // P, P).T
        idxpack[:, 2] = text[b].reshape(NL // P, P).T
        idxpack[:, 3] = res[b].reshape(NL // P, P).T.view(np.int32)
        in_maps.append({
            "adj": np.ascontiguousarray(adj[b]),
            "idxpack": np.ascontiguousarray(idxpack),
            **shared,
        })
    return in_maps


def kernel(**inputs):
    nc = _get_nc()
    in_maps = make_in_maps(inputs)

    from concourse.bass_utils import run_bass_kernel_spmd
    global LAST_RESULT
    LAST_RESULT = run_bass_kernel_spmd(nc, in_maps,
                                       core_ids=list(range(N_CORES)))

    loss = np.zeros([B], np.float32)
    softmax = np.zeros([B, NL], np.float32)
    x = np.zeros([B, NL, D], np.float32)
    for b in range(N_CORES):
        r = LAST_RESULT.results[b]
        loss[b] = r["lossout"][0, 0]
        softmax[b] = r["smout"][0]
        x[b] = r["xout"]
    return loss, softmax, x
